# revision 15
# baseline (speedup 1.0000x reference)
"""Trainium2 Bass kernel for nn_MCPBRNN_Generic_constantoutput_variableLoss_MCA2.

The reference model is a scalar linear recurrence over B=262144 steps:

    ol_t = ol1 * sigmoid(b0 + (u2_t - ML)/SL * wb2)
    f_t  = 1 - oo - ol_t - oogw
    c_{t+1} = f_t * c_t + u1_t          (c_0 = 0)

with per-step outputs using the state BEFORE the step (exclusive scan).
The gates come from a softmax over 4 weights, so f_t is bounded well below 1
(empirically f in [0.41, 0.46] for the generated inputs).  State influence
decays geometrically: after H steps the initial condition contributes at
most f_max^H relative; with H=32 that is < 1e-10, far below f32 resolution.

This lets us break the "strictly sequential" scan into 1024 independent rows
(8 cores x 128 SBUF partitions), each re-running a 32-element warmup halo
from a zero initial state instead of waiting for the true carry.  Each core
runs the native VectorEngine tensor_tensor_scan instruction (state =
f*state + u1 along the free dimension) on a [128, 288] tile: 32 halo + 256
output elements per partition.

Raw Bacc (no TileContext): the kernel is ~25 instructions with hand-placed
semaphores, which avoids Tile's expensive kernel-tail drain + EVSEM barrier
butterfly (~7us) and keeps every instruction at <=1 sync wait (a TRN2
codegen requirement; Bacc's generate_event_semaphores legalizes the rest).

Outputs are packed into 3 fat DRAM tensors (one per producing engine) so
there are only 4 DMAs total; the host slices them back into the 12 module
outputs.  Sharding: core k owns contiguous elements [k*32768, (k+1)*32768).
"""

import os
import sys

import numpy as np

for _p in ("/opt/trn_rl_repo", "/root/.axon_site/_ro/trn_rl_repo"):
    if os.path.isdir(_p) and _p not in sys.path:
        sys.path.append(_p)

B = 262144
N_CORES = 8
CHUNK = B // N_CORES        # 32768 elements per core
ROWS = 128                  # SBUF partitions
ROWLEN = CHUNK // ROWS      # 256 output elements per partition
HALO = 32                   # warmup elements re-scanned from zero state
W = HALO + ROWLEN           # 288 scanned elements per partition
SPIN_LEN = 365
TRAIN_LEN = 200000
ML = 2.9086
SL = 1.898

OUT_NAMES = [
    "h_n", "c_n", "l_n", "gw_n", "bp_n", "gate_ib", "gate_oo",
    "gate_oogw", "gate_ol", "gate_f", "h_nout", "obs_std",
]

# dout_v (DVE-written): [ scan_out(288) | l_n(256) | gate_f(256) | gate_ol(256) ]
#   c_n = scan_out[:, 31:287] (exclusive scan); cols 0..30 and 287 are junk
WV = W + 3 * ROWLEN         # 1056
# dout_a (ACT-written): [ h_n(256) | gw_n(256) | h_nout(512 interleaved) ]
WA = 4 * ROWLEN             # 1024
# dout_c (Pool-written consts): [ bp_n | gate_ib | gate_oo | gate_oogw | obs_std ]
WC = 5 * ROWLEN             # 1280

# Populated after every device run (BassKernelResults); used by test.py for
# profiling. Not used by the grading path.
LAST_RESULTS = None

_BUILD_CACHE = {}


def _make_fast_block(bass_mod, mybir):
    """Block variant with a cheap ending: per-engine drains for the engines
    that did work, and a final barrier that EXCLUDES the (unused) PE engine.
    Walrus appends its ~6us serial semaphore-reset epilogue to PE's stream;
    keeping PE out of the final barrier lets that epilogue run concurrently
    with the kernel instead of after it."""

    class FastBlock(bass_mod.BassBlock):
        def __exit__(self, exc_type, exc_val, exc_tb):
            if exc_type is not None:
                return
            for engine, last_body in self.last_body.items():
                with self.bass.body(
                    last_body, parent=self.bass.cur_bb, allow_existing_parent=True
                ):
                    engine.br(self.end_bb)
            self.bass.switch_bb(self.end_bb)
            barrier_engines = []
            for eng_type, eng in self.bass.engines.items():
                if eng_type == mybir.EngineType.PE:
                    continue
                barrier_engines.append(eng_type)
                if eng_type == mybir.EngineType.Pool:
                    continue  # skip the expensive gpsimd dge_drain (no SWDGE)
                d = mybir.InstDrain(
                    name=self.bass.get_next_instruction_name(),
                    ins=[], outs=[], bass_is_fusable=False,
                )
                d.engine = eng_type
                eng.add_instruction(d)
            self.bass.multi_engine_barrier(barrier_engines)

    from contextlib import contextmanager

    @contextmanager
    def fast_block(nc):
        nc.check_frozen()
        assert nc.cur_block is None
        with FastBlock(nc, f"block_{nc.next_id()}") as blk:
            nc.cur_block = blk
            yield blk
        nc.cur_block = None

    return fast_block


def _build_bass(oo, oogw, ol1, K, sscale, sbias, obsstd):
    import concourse.bacc as bacc
    import concourse.bass as bass_mod
    import concourse.mybir as mybir

    dt = mybir.dt.float32
    AF = mybir.ActivationFunctionType
    OP = mybir.AluOpType

    nc = bacc.Bacc()
    fast_block = _make_fast_block(bass_mod, mybir)
    xh = nc.dram_tensor("xh", [ROWS, 2 * W], dt, kind="ExternalInput")
    dout_v = nc.dram_tensor("dout_v", [ROWS, WV], dt, kind="ExternalOutput")
    dout_a = nc.dram_tensor("dout_a", [ROWS, WA], dt, kind="ExternalOutput")
    dout_c = nc.dram_tensor("dout_c", [ROWS, WC], dt, kind="ExternalOutput")

    R = ROWLEN
    with (
        nc.sbuf_tensor("xt", [ROWS, 2 * W], dt) as xt_h,
        nc.sbuf_tensor("targ", [ROWS, W], dt) as targ_h,
        nc.sbuf_tensor("sig", [ROWS, W], dt) as sig_h,
        nc.sbuf_tensor("ff", [ROWS, W], dt) as ff_h,
        nc.sbuf_tensor("sv", [ROWS, WV], dt) as sv_h,
        nc.sbuf_tensor("sa", [ROWS, WA], dt) as sa_h,
        nc.sbuf_tensor("sc", [ROWS, WC], dt) as sc_h,
        nc.semaphore("s_in") as s_in,
        nc.semaphore("s_v") as s_v,
        nc.semaphore("s_a") as s_a,
        nc.semaphore("s_p") as s_p,
        nc.semaphore("s_out") as s_out,
        fast_block(nc) as block,
    ):
        xt, targ, sig, ff = xt_h[:], targ_h[:], sig_h[:], ff_h[:]
        sv, sa, sc = sv_h[:], sa_h[:], sc_h[:]
        cexcl = sv[:, HALO - 1:HALO - 1 + R]   # c before each output element

        @block.sync
        def _(sp):
            sp.dma_start(out=xt, in_=xh[:]).then_inc(s_in, 16)
            sp.wait_ge(s_p, 4)
            sp.dma_start(out=dout_c[:], in_=sc).then_inc(s_out, 16)
            sp.wait_ge(s_v, 6)
            sp.dma_start(out=dout_v[:], in_=sv).then_inc(s_out, 16)
            sp.wait_ge(s_a, 4)
            sp.dma_start(out=dout_a[:, 0:2 * R],
                         in_=sa[:, 0:2 * R]).then_inc(s_out, 16)
            sp.wait_ge(s_a, 5)
            sp.dma_start(out=dout_a[:, 2 * R:4 * R],
                         in_=sa[:, 2 * R:4 * R]).then_inc(s_out, 16)
            sp.wait_ge(s_out, 64)

        @block.gpsimd
        def _(pool):
            pool.memset(sc[:, 0:2 * R], 0.0).then_inc(s_p, 1)
            pool.memset(sc[:, 2 * R:3 * R], float(oo)).then_inc(s_p, 1)
            pool.memset(sc[:, 3 * R:4 * R], float(oogw)).then_inc(s_p, 1)
            pool.memset(sc[:, 4 * R:5 * R], float(obsstd)).then_inc(s_p, 1)

        @block.vector
        def _(dve):
            dve.wait_ge(s_in, 16)
            # v1: sigmoid argument from the odd (u2) interleaved lane
            dve.tensor_scalar(targ, xt[:, 1::2], float(sscale), float(sbias),
                              OP.mult, OP.add).then_inc(s_v, 1)
            dve.wait_ge(s_a, 1)
            # v2: f = K - ol1*sig
            dve.tensor_scalar(ff, sig, -float(ol1), float(K),
                              OP.mult, OP.add).then_inc(s_v, 1)
            # v3: gate_f = K - ol1*sig on the output window
            dve.tensor_scalar(sv[:, W + R:W + 2 * R], sig[:, HALO:],
                              -float(ol1), float(K),
                              OP.mult, OP.add).then_inc(s_v, 1)
            # v4: gate_ol = ol1*sig on the output window
            dve.tensor_scalar(sv[:, W + 2 * R:W + 3 * R], sig[:, HALO:],
                              float(ol1), None, OP.mult).then_inc(s_v, 1)
            dve.wait_ge(s_v, 2)   # f complete (same-engine RAW)
            # v5: the recurrence itself, written straight into the staging
            # tile; u1 is the even interleaved lane
            dve.tensor_tensor_scan(sv[:, 0:W], ff, xt[:, 0::2], 0.0,
                                   OP.mult, OP.add).then_inc(s_v, 1)
            dve.wait_ge(s_v, 5)   # scan complete
            # v6: l_n = gate_ol * c
            dve.tensor_tensor(sv[:, W:W + R], sv[:, W + 2 * R:W + 3 * R],
                              cexcl, OP.mult).then_inc(s_v, 1)

        @block.scalar
        def _(act):
            act.wait_ge(s_v, 1)
            # a1: sigmoid
            act.activation(sig, targ, AF.Sigmoid).then_inc(s_a, 1)
            # a2: h_nout odd lanes = obs_std (scale=0 copy; input just needs
            # to be any ready tile)
            act.activation(sa[:, 2 * R + 1::2], targ[:, 0:R], AF.Copy,
                           bias=float(obsstd), scale=0.0).then_inc(s_a, 1)
            act.wait_ge(s_v, 5)   # scan complete
            # a3: h_n = oo * c
            act.activation(sa[:, 0:R], cexcl, AF.Copy,
                           bias=0.0, scale=float(oo)).then_inc(s_a, 1)
            # a4: gw_n = oogw * c
            act.activation(sa[:, R:2 * R], cexcl, AF.Copy,
                           bias=0.0, scale=float(oogw)).then_inc(s_a, 1)
            act.wait_ge(s_a, 3)   # h_n complete (same-engine RAW)
            # a5: h_nout even lanes = h_n
            act.activation(sa[:, 2 * R::2], sa[:, 0:R], AF.Copy,
                           bias=0.0, scale=1.0).then_inc(s_a, 1)

    nc.finalize()
    return nc


def _scalars(x, y_obs, weight_r_yom, weight_r_yom_gw, weight_r_ylm,
             weight_r_yfm, bias_b0_ylm, weight_b2_ylm):
    w_om = float(np.asarray(weight_r_yom).reshape(-1)[0])
    w_gw = float(np.asarray(weight_r_yom_gw).reshape(-1)[0])
    w_lm = float(np.asarray(weight_r_ylm).reshape(-1)[0])
    w_fm = float(np.asarray(weight_r_yfm).reshape(-1)[0])
    b0 = float(np.asarray(bias_b0_ylm).reshape(-1)[0])
    wb2 = float(np.asarray(weight_b2_ylm).reshape(-1)[0])

    e = np.exp(np.array([w_om, w_gw, w_lm, w_fm], dtype=np.float64))
    den = float(e.sum())
    oo = float(e[0]) / den
    oogw = float(e[1]) / den
    ol1 = float(e[2]) / den
    K = 1.0 - oo - oogw           # f = K - ol
    sscale = wb2 / SL
    sbias = b0 - ML * wb2 / SL
    obsstd = float(np.std(y_obs[SPIN_LEN:TRAIN_LEN].astype(np.float64),
                          ddof=1))
    return oo, oogw, ol1, K, sscale, sbias, obsstd


def _shard_input(x):
    # per-core inputs: 128 rows of (32 halo + 256) interleaved (u1,u2) pairs;
    # consecutive rows overlap by the halo; core 0's first halo is zeros
    flat = np.concatenate([np.zeros(2 * HALO, np.float32), x.reshape(-1)])
    starts = (np.arange(N_CORES * ROWS, dtype=np.int64) * (2 * ROWLEN))[:, None]
    idx = starts + np.arange(2 * W, dtype=np.int64)[None, :]
    rows = flat[idx].reshape(N_CORES, ROWS, 2 * W)
    return [{"xh": np.ascontiguousarray(rows[k])} for k in range(N_CORES)]


def kernel(x, y_obs, epoch, time_lag, weight_r_yom, weight_r_yom_gw,
           weight_r_ylm, weight_r_yfm, bias_b0_ylm, weight_b2_ylm):
    global LAST_RESULTS
    from concourse.bass_utils import run_bass_kernel_spmd

    x = np.asarray(x, dtype=np.float32)
    y_obs = np.asarray(y_obs, dtype=np.float32)
    oo, oogw, ol1, K, sscale, sbias, obsstd = _scalars(
        x, y_obs, weight_r_yom, weight_r_yom_gw, weight_r_ylm,
        weight_r_yfm, bias_b0_ylm, weight_b2_ylm)

    in_maps = _shard_input(x)

    key = (oo, oogw, ol1, sscale, sbias, obsstd)
    nc = _BUILD_CACHE.get(key)
    if nc is None:
        nc = _build_bass(oo, oogw, ol1, K, sscale, sbias, obsstd)
        _BUILD_CACHE[key] = nc

    res = run_bass_kernel_spmd(nc, in_maps, core_ids=list(range(N_CORES)))
    LAST_RESULTS = res
    results = res.results

    R = ROWLEN
    full = {}
    dv = np.concatenate([results[k]["dout_v"] for k in range(N_CORES)], axis=0)
    da = np.concatenate([results[k]["dout_a"] for k in range(N_CORES)], axis=0)
    dc = np.concatenate([results[k]["dout_c"] for k in range(N_CORES)], axis=0)

    def col(arr, lo, hi):
        return np.ascontiguousarray(arr[:, lo:hi]).reshape(B, 1)

    full["c_n"] = col(dv, HALO - 1, HALO - 1 + R)
    full["l_n"] = col(dv, W, W + R)
    full["gate_f"] = col(dv, W + R, W + 2 * R)
    full["gate_ol"] = col(dv, W + 2 * R, W + 3 * R)
    full["h_n"] = col(da, 0, R)
    full["gw_n"] = col(da, R, 2 * R)
    full["h_nout"] = np.ascontiguousarray(da[:, 2 * R:4 * R]).reshape(B, 2)
    full["bp_n"] = col(dc, 0, R)
    full["gate_ib"] = col(dc, R, 2 * R)
    full["gate_oo"] = col(dc, 2 * R, 3 * R)
    full["gate_oogw"] = col(dc, 3 * R, 4 * R)
    full["obs_std"] = col(dc, 4 * R, 5 * R)
    return tuple(full[name] for name in OUT_NAMES)


# revision 20
# speedup vs baseline: 1.0635x; 1.0635x over previous
"""Trainium2 Bass kernel for nn_MCPBRNN_Generic_constantoutput_variableLoss_MCA2.

The reference model is a scalar linear recurrence over B=262144 steps:

    ol_t = ol1 * sigmoid(b0 + (u2_t - ML)/SL * wb2)
    f_t  = 1 - oo - ol_t - oogw
    c_{t+1} = f_t * c_t + u1_t          (c_0 = 0)

with per-step outputs using the state BEFORE the step (exclusive scan).
The gates come from a softmax over 4 weights, so f_t is bounded well below 1
(empirically f in [0.41, 0.46] for the generated inputs).  State influence
decays geometrically: after H steps the initial condition contributes at
most f_max^H relative; with H=32 that is < 1e-10, far below f32 resolution.

This lets us break the "strictly sequential" scan into 1024 independent rows
(8 cores x 128 SBUF partitions), each re-running a 32-element warmup halo
from a zero initial state instead of waiting for the true carry.  Each core
runs the native VectorEngine tensor_tensor_scan instruction (state =
f*state + u1 along the free dimension) on a [128, 288] tile: 32 halo + 256
output elements per partition.

Raw Bacc (no TileContext): the kernel is ~25 instructions with hand-placed
semaphores, which avoids Tile's expensive kernel-tail drain + EVSEM barrier
butterfly (~7us) and keeps every instruction at <=1 sync wait (a TRN2
codegen requirement; Bacc's generate_event_semaphores legalizes the rest).

Outputs are packed into 3 fat DRAM tensors (one per producing engine) so
there are only 4 DMAs total; the host slices them back into the 12 module
outputs.  Sharding: core k owns contiguous elements [k*32768, (k+1)*32768).
"""

import os
import sys

import numpy as np

for _p in ("/opt/trn_rl_repo", "/root/.axon_site/_ro/trn_rl_repo"):
    if os.path.isdir(_p) and _p not in sys.path:
        sys.path.append(_p)

B = 262144
N_CORES = 8
CHUNK = B // N_CORES        # 32768 elements per core
ROWS = 128                  # SBUF partitions
ROWLEN = CHUNK // ROWS      # 256 output elements per partition
HALO = 32                   # warmup elements re-scanned from zero state
W = HALO + ROWLEN           # 288 scanned elements per partition
SPIN_LEN = 365
TRAIN_LEN = 200000
ML = 2.9086
SL = 1.898

OUT_NAMES = [
    "h_n", "c_n", "l_n", "gw_n", "bp_n", "gate_ib", "gate_oo",
    "gate_oogw", "gate_ol", "gate_f", "h_nout", "obs_std",
]

# dout_v (DVE-written): [ scan(288) | l_n(256) | gate_f(256) | gate_ol(256) | gw_n(256) ]
#   c_n = scan[:, 31:287] (exclusive scan); cols 0..30 and 287 are junk
WV = W + 4 * ROWLEN         # 1312
# dout_a (ACT-written): [ h_n(256) | h_nout(512 interleaved) ]
WA = 3 * ROWLEN             # 768
# dout_c (Pool-written consts): [ bp_n | gate_ib | gate_oo | gate_oogw | obs_std ]
WC = 5 * ROWLEN             # 1280

# Populated after every device run (BassKernelResults); used by test.py for
# profiling. Not used by the grading path.
LAST_RESULTS = None

_BUILD_CACHE = {}


def _make_fast_block(bass_mod, mybir):
    """Block variant with a cheap ending: per-engine drains for the engines
    that did work, and a final barrier that EXCLUDES the (unused) PE engine.
    Walrus appends its ~6us serial semaphore-reset epilogue to PE's stream;
    keeping PE out of the final barrier lets that epilogue run concurrently
    with the kernel instead of after it."""

    class FastBlock(bass_mod.BassBlock):
        def __exit__(self, exc_type, exc_val, exc_tb):
            if exc_type is not None:
                return
            for engine, last_body in self.last_body.items():
                with self.bass.body(
                    last_body, parent=self.bass.cur_bb, allow_existing_parent=True
                ):
                    engine.br(self.end_bb)
            self.bass.switch_bb(self.end_bb)
            barrier_engines = []
            for eng_type, eng in self.bass.engines.items():
                if eng_type == mybir.EngineType.PE:
                    continue
                barrier_engines.append(eng_type)
                if eng_type == mybir.EngineType.Pool:
                    continue  # skip the expensive gpsimd dge_drain (no SWDGE)
                d = mybir.InstDrain(
                    name=self.bass.get_next_instruction_name(),
                    ins=[], outs=[], bass_is_fusable=False,
                )
                d.engine = eng_type
                eng.add_instruction(d)
            self.bass.multi_engine_barrier(barrier_engines)

    from contextlib import contextmanager

    @contextmanager
    def fast_block(nc):
        nc.check_frozen()
        assert nc.cur_block is None
        with FastBlock(nc, f"block_{nc.next_id()}") as blk:
            nc.cur_block = blk
            yield blk
        nc.cur_block = None

    return fast_block


def _build_bass(oo, oogw, ol1, K, sscale, sbias, obsstd):
    import concourse.bacc as bacc
    import concourse.bass as bass_mod
    import concourse.mybir as mybir

    dt = mybir.dt.float32
    AF = mybir.ActivationFunctionType
    OP = mybir.AluOpType

    nc = bacc.Bacc()
    fast_block = _make_fast_block(bass_mod, mybir)
    xh = nc.dram_tensor("xh", [ROWS, 2 * W], dt, kind="ExternalInput")
    dout_v = nc.dram_tensor("dout_v", [ROWS, WV], dt, kind="ExternalOutput")
    dout_a = nc.dram_tensor("dout_a", [ROWS, WA], dt, kind="ExternalOutput")
    dout_c = nc.dram_tensor("dout_c", [ROWS, WC], dt, kind="ExternalOutput")

    R = ROWLEN
    with (
        nc.sbuf_tensor("xt", [ROWS, 2 * W], dt) as xt_h,
        nc.sbuf_tensor("targ", [ROWS, W], dt) as targ_h,
        nc.sbuf_tensor("sig", [ROWS, W], dt) as sig_h,
        nc.sbuf_tensor("ff", [ROWS, W], dt) as ff_h,
        nc.sbuf_tensor("sv", [ROWS, WV], dt) as sv_h,
        nc.sbuf_tensor("sa", [ROWS, WA], dt) as sa_h,
        nc.sbuf_tensor("sc", [ROWS, WC], dt) as sc_h,
        nc.semaphore("s_in") as s_in,
        nc.semaphore("s_v") as s_v,
        nc.semaphore("s_a") as s_a,
        nc.semaphore("s_p") as s_p,
        nc.semaphore("s_out") as s_out,
        fast_block(nc) as block,
    ):
        xt, targ, sig, ff = xt_h[:], targ_h[:], sig_h[:], ff_h[:]
        sv, sa, sc = sv_h[:], sa_h[:], sc_h[:]
        cexcl = sv[:, HALO - 1:HALO - 1 + R]   # c before each output element

        @block.sync
        def _(sp):
            sp.dma_start(out=xt, in_=xh[:]).then_inc(s_in, 16)
            sp.wait_ge(s_p, 4)
            sp.dma_start(out=dout_c[:], in_=sc).then_inc(s_out, 16)
            # the c_n/scan region ships as soon as the scan lands, while the
            # remaining elementwise outputs are still being produced
            sp.wait_ge(s_v, 3)
            sp.dma_start(out=dout_v[:, 0:W], in_=sv[:, 0:W]).then_inc(s_out, 16)
            sp.wait_ge(s_v, 7)
            sp.dma_start(out=dout_v[:, W:WV],
                         in_=sv[:, W:WV]).then_inc(s_out, 16)
            sp.wait_ge(s_a, 4)
            sp.dma_start(out=dout_a[:], in_=sa).then_inc(s_out, 16)
            sp.wait_ge(s_out, 64)

        @block.gpsimd
        def _(pool):
            pool.memset(sc[:, 0:2 * R], 0.0).then_inc(s_p, 1)
            pool.memset(sc[:, 2 * R:3 * R], float(oo)).then_inc(s_p, 1)
            pool.memset(sc[:, 3 * R:4 * R], float(oogw)).then_inc(s_p, 1)
            pool.memset(sc[:, 4 * R:5 * R], float(obsstd)).then_inc(s_p, 1)

        @block.vector
        def _(dve):
            dve.wait_ge(s_in, 16)
            # v1: sigmoid argument from the odd (u2) interleaved lane
            dve.tensor_scalar(targ, xt[:, 1::2], float(sscale), float(sbias),
                              OP.mult, OP.add).then_inc(s_v, 1)
            dve.wait_ge(s_a, 1)
            # v2: f = K - ol1*sig
            dve.tensor_scalar(ff, sig, -float(ol1), float(K),
                              OP.mult, OP.add).then_inc(s_v, 1)
            dve.wait_ge(s_v, 2)   # f complete (same-engine RAW)
            # v3: the recurrence itself, written straight into the staging
            # tile; u1 is the even interleaved lane
            dve.tensor_tensor_scan(sv[:, 0:W], ff, xt[:, 0::2], 0.0,
                                   OP.mult, OP.add).then_inc(s_v, 1)
            # v4: gate_ol = ol1*sig on the output window
            dve.tensor_scalar(sv[:, W + 2 * R:W + 3 * R], sig[:, HALO:],
                              float(ol1), None, OP.mult).then_inc(s_v, 1)
            dve.wait_ge(s_v, 4)   # scan + gate_ol complete (same-engine RAW)
            # v5: l_n = gate_ol * c
            dve.tensor_tensor(sv[:, W:W + R], sv[:, W + 2 * R:W + 3 * R],
                              cexcl, OP.mult).then_inc(s_v, 1)
            # v6: gate_f = K - ol1*sig on the output window
            dve.tensor_scalar(sv[:, W + R:W + 2 * R], sig[:, HALO:],
                              -float(ol1), float(K),
                              OP.mult, OP.add).then_inc(s_v, 1)
            # v7: gw_n = oogw * c
            dve.tensor_scalar(sv[:, W + 3 * R:W + 4 * R], cexcl,
                              float(oogw), None, OP.mult).then_inc(s_v, 1)

        @block.scalar
        def _(act):
            act.wait_ge(s_v, 1)
            # a1: sigmoid
            act.activation(sig, targ, AF.Sigmoid).then_inc(s_a, 1)
            # a2: h_nout odd lanes = obs_std (scale=0 copy; input just needs
            # to be any ready tile)
            act.activation(sa[:, R + 1::2], targ[:, 0:R], AF.Copy,
                           bias=float(obsstd), scale=0.0).then_inc(s_a, 1)
            act.wait_ge(s_v, 3)   # scan complete
            # a3: h_n = oo * c
            act.activation(sa[:, 0:R], cexcl, AF.Copy,
                           bias=0.0, scale=float(oo)).then_inc(s_a, 1)
            act.wait_ge(s_a, 3)   # h_n complete (same-engine RAW)
            # a4: h_nout even lanes = h_n
            act.activation(sa[:, R::2], sa[:, 0:R], AF.Copy,
                           bias=0.0, scale=1.0).then_inc(s_a, 1)

    nc.finalize()
    return nc


def _scalars(x, y_obs, weight_r_yom, weight_r_yom_gw, weight_r_ylm,
             weight_r_yfm, bias_b0_ylm, weight_b2_ylm):
    w_om = float(np.asarray(weight_r_yom).reshape(-1)[0])
    w_gw = float(np.asarray(weight_r_yom_gw).reshape(-1)[0])
    w_lm = float(np.asarray(weight_r_ylm).reshape(-1)[0])
    w_fm = float(np.asarray(weight_r_yfm).reshape(-1)[0])
    b0 = float(np.asarray(bias_b0_ylm).reshape(-1)[0])
    wb2 = float(np.asarray(weight_b2_ylm).reshape(-1)[0])

    e = np.exp(np.array([w_om, w_gw, w_lm, w_fm], dtype=np.float64))
    den = float(e.sum())
    oo = float(e[0]) / den
    oogw = float(e[1]) / den
    ol1 = float(e[2]) / den
    K = 1.0 - oo - oogw           # f = K - ol
    sscale = wb2 / SL
    sbias = b0 - ML * wb2 / SL
    obsstd = float(np.std(y_obs[SPIN_LEN:TRAIN_LEN].astype(np.float64),
                          ddof=1))
    return oo, oogw, ol1, K, sscale, sbias, obsstd


def _shard_input(x):
    # per-core inputs: 128 rows of (32 halo + 256) interleaved (u1,u2) pairs;
    # consecutive rows overlap by the halo; core 0's first halo is zeros
    flat = np.concatenate([np.zeros(2 * HALO, np.float32), x.reshape(-1)])
    starts = (np.arange(N_CORES * ROWS, dtype=np.int64) * (2 * ROWLEN))[:, None]
    idx = starts + np.arange(2 * W, dtype=np.int64)[None, :]
    rows = flat[idx].reshape(N_CORES, ROWS, 2 * W)
    return [{"xh": np.ascontiguousarray(rows[k])} for k in range(N_CORES)]


def kernel(x, y_obs, epoch, time_lag, weight_r_yom, weight_r_yom_gw,
           weight_r_ylm, weight_r_yfm, bias_b0_ylm, weight_b2_ylm):
    global LAST_RESULTS
    from concourse.bass_utils import run_bass_kernel_spmd

    x = np.asarray(x, dtype=np.float32)
    y_obs = np.asarray(y_obs, dtype=np.float32)
    oo, oogw, ol1, K, sscale, sbias, obsstd = _scalars(
        x, y_obs, weight_r_yom, weight_r_yom_gw, weight_r_ylm,
        weight_r_yfm, bias_b0_ylm, weight_b2_ylm)

    in_maps = _shard_input(x)

    key = (oo, oogw, ol1, sscale, sbias, obsstd)
    nc = _BUILD_CACHE.get(key)
    if nc is None:
        nc = _build_bass(oo, oogw, ol1, K, sscale, sbias, obsstd)
        _BUILD_CACHE[key] = nc

    res = run_bass_kernel_spmd(nc, in_maps, core_ids=list(range(N_CORES)))
    LAST_RESULTS = res
    results = res.results

    R = ROWLEN
    full = {}
    dv = np.concatenate([results[k]["dout_v"] for k in range(N_CORES)], axis=0)
    da = np.concatenate([results[k]["dout_a"] for k in range(N_CORES)], axis=0)
    dc = np.concatenate([results[k]["dout_c"] for k in range(N_CORES)], axis=0)

    def col(arr, lo, hi):
        return np.ascontiguousarray(arr[:, lo:hi]).reshape(B, 1)

    full["c_n"] = col(dv, HALO - 1, HALO - 1 + R)
    full["l_n"] = col(dv, W, W + R)
    full["gate_f"] = col(dv, W + R, W + 2 * R)
    full["gate_ol"] = col(dv, W + 2 * R, W + 3 * R)
    full["gw_n"] = col(dv, W + 3 * R, W + 4 * R)
    full["h_n"] = col(da, 0, R)
    full["h_nout"] = np.ascontiguousarray(da[:, R:3 * R]).reshape(B, 2)
    full["bp_n"] = col(dc, 0, R)
    full["gate_ib"] = col(dc, R, 2 * R)
    full["gate_oo"] = col(dc, 2 * R, 3 * R)
    full["gate_oogw"] = col(dc, 3 * R, 4 * R)
    full["obs_std"] = col(dc, 4 * R, 5 * R)
    return tuple(full[name] for name in OUT_NAMES)


# revision 23
# speedup vs baseline: 1.2526x; 1.1778x over previous
"""Trainium2 Bass kernel for nn_MCPBRNN_Generic_constantoutput_variableLoss_MCA2.

The reference model is a scalar linear recurrence over B=262144 steps:

    ol_t = ol1 * sigmoid(b0 + (u2_t - ML)/SL * wb2)
    f_t  = 1 - oo - ol_t - oogw
    c_{t+1} = f_t * c_t + u1_t          (c_0 = 0)

with per-step outputs using the state BEFORE the step (exclusive scan).
The gates come from a softmax over 4 weights, so f_t is bounded well below 1
(empirically f in [0.41, 0.46] for the generated inputs).  State influence
decays geometrically: after H steps the initial condition contributes at
most f_max^H relative; with H=32 that is < 1e-10, far below f32 resolution.

This lets us break the "strictly sequential" scan into 1024 independent rows
(8 cores x 128 SBUF partitions), each re-running a 32-element warmup halo
from a zero initial state instead of waiting for the true carry.  Each core
runs the native VectorEngine tensor_tensor_scan instruction (state =
f*state + u1 along the free dimension) on a [128, 288] tile: 32 halo + 256
output elements per partition.

Raw Bacc (no TileContext): the kernel is ~25 instructions with hand-placed
semaphores, which avoids Tile's expensive kernel-tail drain + EVSEM barrier
butterfly (~7us) and keeps every instruction at <=1 sync wait (a TRN2
codegen requirement; Bacc's generate_event_semaphores legalizes the rest).

Outputs are packed into 3 fat DRAM tensors (one per producing engine) so
there are only 4 DMAs total; the host slices them back into the 12 module
outputs.  Sharding: core k owns contiguous elements [k*32768, (k+1)*32768).
"""

import os
import sys

import numpy as np

for _p in ("/opt/trn_rl_repo", "/root/.axon_site/_ro/trn_rl_repo"):
    if os.path.isdir(_p) and _p not in sys.path:
        sys.path.append(_p)

B = 262144
N_CORES = 8
CHUNK = B // N_CORES        # 32768 elements per core
ROWS = 128                  # SBUF partitions
ROWLEN = CHUNK // ROWS      # 256 output elements per partition
HALO = 32                   # warmup elements re-scanned from zero state
W = HALO + ROWLEN           # 288 scanned elements per partition
SPIN_LEN = 365
TRAIN_LEN = 200000
ML = 2.9086
SL = 1.898

OUT_NAMES = [
    "h_n", "c_n", "l_n", "gw_n", "bp_n", "gate_ib", "gate_oo",
    "gate_oogw", "gate_ol", "gate_f", "h_nout", "obs_std",
]

# dout_v (DVE-written): [ scan(288) | f(288) | l_n(256) | gate_ol(256) | gw_n(256) ]
#   c_n = scan[:, 31:287] (exclusive scan; cols 0..30, 287 junk)
#   gate_f = f[:, 32:288] (the f tile doubles as the gate_f output window)
WV = 2 * W + 3 * ROWLEN     # 1344
# dout_a (ACT-written): h_nout(512 interleaved); h_n and obs_std are its
# even/odd lanes, so the host derives them instead of writing them twice
WA = 2 * ROWLEN             # 512
# dout_c (Pool-written consts): [ bp_n | gate_ib | gate_oo | gate_oogw ]
WC = 4 * ROWLEN             # 1024

# Populated after every device run (BassKernelResults); used by test.py for
# profiling. Not used by the grading path.
LAST_RESULTS = None

_BUILD_CACHE = {}


def _make_fast_block(bass_mod, mybir):
    """Block variant with a cheap ending: per-engine drains for the engines
    that did work, and a final barrier that EXCLUDES the (unused) PE engine.
    Walrus appends its ~6us serial semaphore-reset epilogue to PE's stream;
    keeping PE out of the final barrier lets that epilogue run concurrently
    with the kernel instead of after it."""

    class FastBlock(bass_mod.BassBlock):
        def __exit__(self, exc_type, exc_val, exc_tb):
            if exc_type is not None:
                return
            for engine, last_body in self.last_body.items():
                with self.bass.body(
                    last_body, parent=self.bass.cur_bb, allow_existing_parent=True
                ):
                    engine.br(self.end_bb)
            self.bass.switch_bb(self.end_bb)
            barrier_engines = []
            for eng_type, eng in self.bass.engines.items():
                if eng_type == mybir.EngineType.PE:
                    continue
                barrier_engines.append(eng_type)
                if eng_type == mybir.EngineType.Pool:
                    continue  # skip the expensive gpsimd dge_drain (no SWDGE)
                d = mybir.InstDrain(
                    name=self.bass.get_next_instruction_name(),
                    ins=[], outs=[], bass_is_fusable=False,
                )
                d.engine = eng_type
                eng.add_instruction(d)
            self.bass.multi_engine_barrier(barrier_engines)

    from contextlib import contextmanager

    @contextmanager
    def fast_block(nc):
        nc.check_frozen()
        assert nc.cur_block is None
        with FastBlock(nc, f"block_{nc.next_id()}") as blk:
            nc.cur_block = blk
            yield blk
        nc.cur_block = None

    return fast_block


def _build_bass(oo, oogw, ol1, K, sscale, sbias, obsstd):
    import concourse.bacc as bacc
    import concourse.bass as bass_mod
    import concourse.mybir as mybir

    dt = mybir.dt.float32
    AF = mybir.ActivationFunctionType
    OP = mybir.AluOpType

    nc = bacc.Bacc()
    fast_block = _make_fast_block(bass_mod, mybir)
    xh = nc.dram_tensor("xh", [ROWS, 2 * W], dt, kind="ExternalInput")
    dout_v = nc.dram_tensor("dout_v", [ROWS, WV], dt, kind="ExternalOutput")
    dout_a = nc.dram_tensor("dout_a", [ROWS, WA], dt, kind="ExternalOutput")
    dout_c = nc.dram_tensor("dout_c", [ROWS, WC], dt, kind="ExternalOutput")

    R = ROWLEN
    with (
        nc.sbuf_tensor("xt", [ROWS, 2 * W], dt) as xt_h,
        nc.sbuf_tensor("targ", [ROWS, W], dt) as targ_h,
        nc.sbuf_tensor("sig", [ROWS, W], dt) as sig_h,
        nc.sbuf_tensor("sv", [ROWS, WV], dt) as sv_h,
        nc.sbuf_tensor("sa", [ROWS, WA], dt) as sa_h,
        nc.sbuf_tensor("sc", [ROWS, WC], dt) as sc_h,
        nc.semaphore("s_in") as s_in,
        nc.semaphore("s_v") as s_v,
        nc.semaphore("s_a") as s_a,
        nc.semaphore("s_p") as s_p,
        nc.semaphore("s_out") as s_out,
        fast_block(nc) as block,
    ):
        xt, targ, sig = xt_h[:], targ_h[:], sig_h[:]
        sv, sa, sc = sv_h[:], sa_h[:], sc_h[:]
        cexcl = sv[:, HALO - 1:HALO - 1 + R]   # c before each output element
        ff = sv[:, W:2 * W]                    # f; [32:] is the gate_f output

        @block.sync
        def _(sp):
            sp.dma_start(out=xt, in_=xh[:]).then_inc(s_in, 16)
            sp.wait_ge(s_p, 3)
            sp.dma_start(out=dout_c[:], in_=sc).then_inc(s_out, 16)
            # scan + f regions (c_n, gate_f) ship as soon as the scan lands,
            # while the remaining elementwise outputs are still being made
            sp.wait_ge(s_v, 3)
            sp.dma_start(out=dout_v[:, 0:2 * W],
                         in_=sv[:, 0:2 * W]).then_inc(s_out, 16)
            sp.wait_ge(s_v, 6)
            sp.dma_start(out=dout_v[:, 2 * W:WV],
                         in_=sv[:, 2 * W:WV]).then_inc(s_out, 16)
            sp.wait_ge(s_a, 3)
            sp.dma_start(out=dout_a[:], in_=sa).then_inc(s_out, 16)
            # no s_out wait: the SP drain at block exit gates DMA completion

        @block.gpsimd
        def _(pool):
            pool.memset(sc[:, 0:2 * R], 0.0).then_inc(s_p, 1)
            pool.memset(sc[:, 2 * R:3 * R], float(oo)).then_inc(s_p, 1)
            pool.memset(sc[:, 3 * R:4 * R], float(oogw)).then_inc(s_p, 1)

        @block.vector
        def _(dve):
            dve.wait_ge(s_in, 16)
            # v1: sigmoid argument from the odd (u2) interleaved lane
            dve.tensor_scalar(targ, xt[:, 1::2], float(sscale), float(sbias),
                              OP.mult, OP.add).then_inc(s_v, 1)
            dve.wait_ge(s_a, 1)
            # v2: f = K - ol1*sig, straight into the staging tile
            dve.tensor_scalar(ff, sig, -float(ol1), float(K),
                              OP.mult, OP.add).then_inc(s_v, 1)
            dve.wait_ge(s_v, 2)   # f complete (same-engine RAW)
            # v3: the recurrence itself, written straight into the staging
            # tile; u1 is the even interleaved lane
            dve.tensor_tensor_scan(sv[:, 0:W], ff, xt[:, 0::2], 0.0,
                                   OP.mult, OP.add).then_inc(s_v, 1)
            # v4: gate_ol = ol1*sig on the output window
            dve.tensor_scalar(sv[:, 2 * W + R:2 * W + 2 * R], sig[:, HALO:],
                              float(ol1), None, OP.mult).then_inc(s_v, 1)
            dve.wait_ge(s_v, 4)   # scan + gate_ol complete (same-engine RAW)
            # v5: l_n = gate_ol * c
            dve.tensor_tensor(sv[:, 2 * W:2 * W + R],
                              sv[:, 2 * W + R:2 * W + 2 * R],
                              cexcl, OP.mult).then_inc(s_v, 1)
            # v6: gw_n = oogw * c
            dve.tensor_scalar(sv[:, 2 * W + 2 * R:2 * W + 3 * R], cexcl,
                              float(oogw), None, OP.mult).then_inc(s_v, 1)

        @block.scalar
        def _(act):
            act.wait_ge(s_v, 1)
            # a1: sigmoid
            act.activation(sig, targ, AF.Sigmoid).then_inc(s_a, 1)
            # a2: h_nout odd lanes = obs_std (scale=0 copy; input just needs
            # to be any ready tile)
            act.activation(sa[:, 1::2], targ[:, 0:R], AF.Copy,
                           bias=float(obsstd), scale=0.0).then_inc(s_a, 1)
            act.wait_ge(s_v, 3)   # scan complete
            # a3: h_nout even lanes = h_n = oo * c (h_n itself is derived
            # from these lanes on the host)
            act.activation(sa[:, 0::2], cexcl, AF.Copy,
                           bias=0.0, scale=float(oo)).then_inc(s_a, 1)

    nc.finalize()
    return nc


def _scalars(x, y_obs, weight_r_yom, weight_r_yom_gw, weight_r_ylm,
             weight_r_yfm, bias_b0_ylm, weight_b2_ylm):
    w_om = float(np.asarray(weight_r_yom).reshape(-1)[0])
    w_gw = float(np.asarray(weight_r_yom_gw).reshape(-1)[0])
    w_lm = float(np.asarray(weight_r_ylm).reshape(-1)[0])
    w_fm = float(np.asarray(weight_r_yfm).reshape(-1)[0])
    b0 = float(np.asarray(bias_b0_ylm).reshape(-1)[0])
    wb2 = float(np.asarray(weight_b2_ylm).reshape(-1)[0])

    e = np.exp(np.array([w_om, w_gw, w_lm, w_fm], dtype=np.float64))
    den = float(e.sum())
    oo = float(e[0]) / den
    oogw = float(e[1]) / den
    ol1 = float(e[2]) / den
    K = 1.0 - oo - oogw           # f = K - ol
    sscale = wb2 / SL
    sbias = b0 - ML * wb2 / SL
    obsstd = float(np.std(y_obs[SPIN_LEN:TRAIN_LEN].astype(np.float64),
                          ddof=1))
    return oo, oogw, ol1, K, sscale, sbias, obsstd


def _shard_input(x):
    # per-core inputs: 128 rows of (32 halo + 256) interleaved (u1,u2) pairs;
    # consecutive rows overlap by the halo; core 0's first halo is zeros
    flat = np.concatenate([np.zeros(2 * HALO, np.float32), x.reshape(-1)])
    starts = (np.arange(N_CORES * ROWS, dtype=np.int64) * (2 * ROWLEN))[:, None]
    idx = starts + np.arange(2 * W, dtype=np.int64)[None, :]
    rows = flat[idx].reshape(N_CORES, ROWS, 2 * W)
    return [{"xh": np.ascontiguousarray(rows[k])} for k in range(N_CORES)]


def kernel(x, y_obs, epoch, time_lag, weight_r_yom, weight_r_yom_gw,
           weight_r_ylm, weight_r_yfm, bias_b0_ylm, weight_b2_ylm):
    global LAST_RESULTS
    from concourse.bass_utils import run_bass_kernel_spmd

    x = np.asarray(x, dtype=np.float32)
    y_obs = np.asarray(y_obs, dtype=np.float32)
    oo, oogw, ol1, K, sscale, sbias, obsstd = _scalars(
        x, y_obs, weight_r_yom, weight_r_yom_gw, weight_r_ylm,
        weight_r_yfm, bias_b0_ylm, weight_b2_ylm)

    in_maps = _shard_input(x)

    key = (oo, oogw, ol1, sscale, sbias, obsstd)
    nc = _BUILD_CACHE.get(key)
    if nc is None:
        nc = _build_bass(oo, oogw, ol1, K, sscale, sbias, obsstd)
        _BUILD_CACHE[key] = nc

    res = run_bass_kernel_spmd(nc, in_maps, core_ids=list(range(N_CORES)))
    LAST_RESULTS = res
    results = res.results

    R = ROWLEN
    full = {}
    dv = np.concatenate([results[k]["dout_v"] for k in range(N_CORES)], axis=0)
    da = np.concatenate([results[k]["dout_a"] for k in range(N_CORES)], axis=0)
    dc = np.concatenate([results[k]["dout_c"] for k in range(N_CORES)], axis=0)

    def col(arr, lo, hi):
        return np.ascontiguousarray(arr[:, lo:hi]).reshape(B, 1)

    full["c_n"] = col(dv, HALO - 1, HALO - 1 + R)
    full["gate_f"] = col(dv, W + HALO, 2 * W)
    full["l_n"] = col(dv, 2 * W, 2 * W + R)
    full["gate_ol"] = col(dv, 2 * W + R, 2 * W + 2 * R)
    full["gw_n"] = col(dv, 2 * W + 2 * R, 2 * W + 3 * R)
    hno = np.ascontiguousarray(da).reshape(B, 2)
    full["h_nout"] = hno
    full["h_n"] = np.ascontiguousarray(hno[:, 0:1])
    full["obs_std"] = np.ascontiguousarray(hno[:, 1:2])
    full["bp_n"] = col(dc, 0, R)
    full["gate_ib"] = col(dc, R, 2 * R)
    full["gate_oo"] = col(dc, 2 * R, 3 * R)
    full["gate_oogw"] = col(dc, 3 * R, 4 * R)
    return tuple(full[name] for name in OUT_NAMES)
